# revision 19
# baseline (speedup 1.0000x reference)
"""BailingMoE block (router + 16 routed experts top-4 + shared SwiGLU MLP)
as a Trainium2 Bass/Tile kernel, expert-parallel over 8 NeuronCores.

Sharding:
  - Routed expert weight stacks [E,H,I] split along E: 2 experts per core
    (cast to bf16 on host; fp32 PSUM accumulation on device).
  - Shared-expert MLP tensor-parallel along the intermediate dim: 128 of
    1024 shared-intermediate channels per core.
  - Router replicated (fp32 - top-4 selection must match the reference);
    per-core the router weight columns are permuted so that each core's own
    2 experts land in columns 0/1 (softmax/top-k are permutation invariant).
  - Each core produces a full [T,H] partial (its experts + its shared
    slice); four bf16 ReduceScatters (256-token chunks, interleaved with
    the down-proj blocks so the collective overlaps compute) sum the
    partials on-device; each core casts its disjoint token slices back
    to fp32, and the host concatenates (pure unshard, no host math).
  - Router matmuls run as float32r (1 cycle/row on PE vs 4 for fp32);
    top-4 selection is bit-identical to fp32 for these logits.

Device dataflow (per core, all matmuls bf16 with fp32 accumulation):
  Xt = X^T staged [H,T];  G_t/U_t = Wg^T X^T per expert in [I,T] layout so
  both operands of every matmul are in their native layout (no transposes
  on the heavy path);  H = silu(G)*U*combine;  down-proj accumulates both
  experts + shared slice into one PSUM tile per (token-tile, out-half).
"""

import numpy as np
import ml_dtypes

import concourse.bass as bass
import concourse.mybir as mybir
import concourse.tile as tile
from concourse import bacc
from concourse.bass_utils import run_bass_kernel_spmd
from concourse.masks import make_identity

BF16 = ml_dtypes.bfloat16

NCORES = 8
T = 1024
H = 1024
I = 512  # routed expert intermediate
E = 16
TOP_K = 4
E_LOC = 2  # experts per core
ISH = 128  # shared-intermediate channels per core (1024 / 8)
KT = H // 128  # 8 contraction tiles over H
NI = I // 128  # 4 partition tiles over I
NTT = T // 128  # 8 token tiles
NCHUNK = 2  # ReduceScatter chunks

F32 = mybir.dt.float32
BF = mybir.dt.bfloat16


def build_nc():
    nc = bacc.Bacc("TRN2", target_bir_lowering=False, debug=False,
                   num_devices=NCORES)

    xtf = nc.dram_tensor("xtf", [H, T], mybir.dt.float32r, kind="ExternalInput")
    xtb = nc.dram_tensor("xtb", [H, T], BF, kind="ExternalInput")
    rwt = nc.dram_tensor("rwt", [128, KT * E], mybir.dt.float32r, kind="ExternalInput")
    wg = nc.dram_tensor("wg", [E_LOC, H, I], BF, kind="ExternalInput")
    wu = nc.dram_tensor("wu", [E_LOC, H, I], BF, kind="ExternalInput")
    wd = nc.dram_tensor("wd", [E_LOC, I, H], BF, kind="ExternalInput")
    wsgu = nc.dram_tensor("wsgu", [H, 2 * ISH], BF, kind="ExternalInput")
    wsd = nc.dram_tensor("wsd", [ISH, H], BF, kind="ExternalInput")
    o = nc.dram_tensor("o", [NCHUNK * (T // NCHUNK // NCORES), H], F32,
                       kind="ExternalOutput")

    rg = [list(range(NCORES))]

    with tile.TileContext(nc) as tc:
        with (
            tc.tile_pool(name="big", bufs=1) as big,
            tc.tile_pool(name="small", bufs=3) as small,
            tc.tile_pool(name="gs_pool", bufs=3) as gs_pool,
            tc.tile_pool(name="accs", bufs=3) as accs,
            tc.tile_pool(name="ps_small", bufs=1, space="PSUM") as ps_small,
            tc.tile_pool(name="ps_gu", bufs=2, space="PSUM") as ps_gu,
            tc.tile_pool(name="ps_acc", bufs=3, space="PSUM") as ps_acc,
            tc.tile_pool(name="dram", bufs=1, space="DRAM") as dram,
        ):
            # ---- staged inputs (everything fits in SBUF); DMAs chunked and
            # emitted in consumption order so compute starts early: the
            # shared-expert + expert-0 gate/up weights and X-bf16 come first
            # (first PE work), the fp32 X for the router after ----
            rwt_sb = big.tile([128, KT, E], mybir.dt.float32r)
            nc.sync.dma_start(out=rwt_sb, in_=rwt.rearrange("p (k e) -> p k e", e=E))
            wsgu_sb = big.tile([128, KT, 2 * ISH], BF)
            nc.sync.dma_start(out=wsgu_sb, in_=wsgu.rearrange("(k p) i -> p k i", p=128))
            xtb_r = xtb.rearrange("(k p) t -> p k t", p=128)
            xtb_sb = big.tile([128, KT, T], BF)
            for k in range(KT):
                nc.sync.dma_start(out=xtb_sb[:, k, :], in_=xtb_r[:, k, :])
            wg_sb = big.tile([128, E_LOC, KT, I], BF)
            wu_sb = big.tile([128, E_LOC, KT, I], BF)
            wg_r = wg.rearrange("e (k p) i -> p e k i", p=128)
            wu_r = wu.rearrange("e (k p) i -> p e k i", p=128)
            nc.sync.dma_start(out=wg_sb[:, 0], in_=wg_r[:, 0])
            nc.sync.dma_start(out=wu_sb[:, 0], in_=wu_r[:, 0])
            xtf_r = xtf.rearrange("(k p) t -> p k t", p=128)
            xtf_sb = big.tile([128, KT, T], mybir.dt.float32r)
            for k in range(KT):
                for hhalf in range(2):
                    hsl2 = slice(hhalf * 512, (hhalf + 1) * 512)
                    nc.sync.dma_start(out=xtf_sb[:, k, hsl2],
                                      in_=xtf_r[:, k, hsl2])
            nc.sync.dma_start(out=wg_sb[:, 1], in_=wg_r[:, 1])
            nc.sync.dma_start(out=wu_sb[:, 1], in_=wu_r[:, 1])
            wd_sb = big.tile([128, E_LOC, NI, H], BF)
            nc.sync.dma_start(out=wd_sb, in_=wd.rearrange("e (n p) h -> p e n h", p=128))
            wsd_sb = big.tile([128, H], BF)
            nc.sync.dma_start(out=wsd_sb, in_=wsd[:])

            identity = big.tile([128, 128], F32)
            make_identity(nc, identity)

            # one DRAM tensor per chunk: a shared tensor would put a false
            # WAR dependency between chunk k's RS read and chunk k+1's writes.
            # RS payload is bf16 (halves collective bytes); output cast back
            # to fp32 on-device after the RS.
            acc_dram = [dram.tile([T // NCHUNK, H], BF, name=f"acc_dram{i}")
                        for i in range(NCHUNK)]
            rs_out = dram.tile([NCHUNK, T // NCHUNK // NCORES, H], BF)
            c_scr = dram.tile([E_LOC, T], BF)


            # ---- router PE part: logits computed transposed ([E,T]:
            # 16 N=512 fp32 matmuls beat 64 N=16 ones), PE-transposed back
            # per token tile into [128, 8, 16] ----
            lgt_sb = small.tile([E, T], F32)

            def router_logits():
                for th in range(2):
                    tsl = slice(th * 512, (th + 1) * 512)
                    lgt_ps = ps_small.tile([E, 512], F32, tag="sm", name="lgt_ps")
                    for k in range(KT):
                        nc.tensor.matmul(lgt_ps[:], rwt_sb[:, k, :],
                                         xtf_sb[:, k, tsl],
                                         start=(k == 0), stop=(k == KT - 1))
                    nc.scalar.copy(lgt_sb[:, tsl], lgt_ps[:])

            L = small.tile([128, NTT, E], F32)

            def router_transposes():
                for tt in range(NTT):
                    tr_ps = ps_acc.tile([128, E], F32, tag="acc", name="tr_ps")
                    nc.tensor.transpose(tr_ps[:],
                                        lgt_sb[:, tt * 128:(tt + 1) * 128],
                                        identity[0:E, 0:E])
                    nc.scalar.copy(L[:, tt, :], tr_ps[:])

            h_sb = big.tile([128, E_LOC, NI, T], BF)
            hs_sb = big.tile([128, T], BF)
            c_sb = big.tile([128, E_LOC, T], BF)
            shard = T // NCHUNK // NCORES  # 64

            def gu_block(e, th):
                """G/U projections + H = silu(G)*U for one expert/half
                (e is None -> shared-expert slice)."""
                tsl = slice(th * 512, (th + 1) * 512)
                for ni in range(NI if e is not None else 1):
                    g_ps = ps_gu.tile([128, 512], F32, tag="g", name="g_ps",
                                      bufs=2)
                    u_ps = ps_gu.tile([128, 512], F32, tag="u", name="u_ps")
                    if e is None:
                        wgl = wsgu_sb[:, :, 0:ISH]
                        wul = wsgu_sb[:, :, ISH:2 * ISH]
                        isl = slice(0, ISH)
                    else:
                        wgl = wg_sb[:, e]
                        wul = wu_sb[:, e]
                        isl = slice(ni * 128, (ni + 1) * 128)
                    for k in range(KT):
                        nc.tensor.matmul(g_ps[:], wgl[:, k, isl],
                                         xtb_sb[:, k, tsl],
                                         start=(k == 0), stop=(k == KT - 1))
                    for k in range(KT):
                        nc.tensor.matmul(u_ps[:], wul[:, k, isl],
                                         xtb_sb[:, k, tsl],
                                         start=(k == 0), stop=(k == KT - 1))
                    gs = gs_pool.tile([128, 512], BF, tag="gs", name="gs")
                    nc.scalar.activation(gs[:], g_ps[:],
                                         mybir.ActivationFunctionType.Silu)
                    dst = hs_sb[:, tsl] if e is None else h_sb[:, e, ni, tsl]
                    nc.vector.tensor_mul(dst, gs[:], u_ps[:])
                    if e is not None and th == 1:
                        # second half: the combine weights are long since
                        # ready - scale inline instead of a deferred pass
                        nc.vector.tensor_mul(dst, dst, c_sb[:, e, tsl])

            def router_chain():
                """Top-4 + normalized combine weights. The per-token-tile
                top-8 reductions (nc.vector.max) are independent, so they
                pipeline on DVE instead of the serial 4-round extraction."""
                m = small.tile([128, NTT, 8], F32)
                msk = small.tile([128, NTT, E], F32)
                for tt in range(NTT):
                    nc.vector.max(m[:, tt, :], L[:, tt, :])
                nc.vector.tensor_tensor(
                    msk[:], L[:], m[:, :, 3:4].to_broadcast([128, NTT, E]),
                    op=mybir.AluOpType.is_ge)
                nc.vector.tensor_tensor(
                    L[:], L[:], m[:, :, 0:1].to_broadcast([128, NTT, E]),
                    op=mybir.AluOpType.subtract)
                nc.scalar.activation(L[:], L[:],
                                     mybir.ActivationFunctionType.Exp)
                nc.vector.tensor_mul(L[:], L[:], msk[:])
                ssum = small.tile([128, NTT, 1], F32)
                nc.vector.reduce_sum(ssum[:, :, 0], L[:],
                                     axis=mybir.AxisListType.X)
                rcp = small.tile([128, NTT, 1], F32)
                nc.vector.reciprocal(rcp[:, :, 0], ssum[:, :, 0])
                nc.vector.tensor_mul(L[:], L[:],
                                     rcp[:].to_broadcast([128, NTT, E]))

            def chain_tail():
                # one PE transpose: [tok128, (tt e)] -> [(tt e), tok128]
                ct_ps = ps_small.tile([128, 128], F32, tag="sm")
                nc.tensor.transpose(ct_ps[:], L.rearrange("p t e -> p (t e)"),
                                    identity[:])
                ct_sb = small.tile([128, 128], BF)
                nc.scalar.copy(ct_sb[:], ct_ps[:])
                # per-expert combine rows broadcast to 128 partitions via a
                # DRAM round-trip (the DMA replicates the row)
                ct_v = ct_sb.rearrange("(t e) x -> t e x", e=E)
                for e in range(E_LOC):
                    nc.sync.dma_start(
                        out=c_scr[e].rearrange("(t x) -> t x", x=128),
                        in_=ct_v[:, e, :])
                    nc.sync.dma_start(
                        out=c_sb[:, e, :],
                        in_=c_scr[e:e + 1, :].to_broadcast([128, T]))

            def scale_block(th):
                # deferred combine scale (first half only: its mul1s run
                # before the router chain finishes)
                if th == 1:
                    return
                tsl = slice(th * 512, (th + 1) * 512)
                for e in range(E_LOC):
                    for ni in range(NI):
                        nc.vector.tensor_mul(h_sb[:, e, ni, tsl],
                                             h_sb[:, e, ni, tsl],
                                             c_sb[:, e, tsl])

            def down_block(th):
                chunks_per_th = NCHUNK // 2
                tt_per_chunk = NTT // NCHUNK
                for cq in range(chunks_per_th):
                    chunk = th * chunks_per_th + cq
                    for ti in range(tt_per_chunk):
                        tt = chunk * tt_per_chunk + ti
                        dsl = slice(tt * 128, (tt + 1) * 128)
                        lsl = slice(ti * 128, (ti + 1) * 128)
                        # both output halves interleaved: each stationary
                        # H-tile load feeds two matmuls back-to-back
                        acc_a = ps_acc.tile([128, 512], F32, tag="acc",
                                            name="acc_a")
                        acc_b = ps_acc.tile([128, 512], F32, tag="acc",
                                            name="acc_b")
                        first = True
                        for e in range(E_LOC):
                            for ni in range(NI):
                                nc.tensor.matmul(acc_a[:],
                                                 h_sb[:, e, ni, dsl],
                                                 wd_sb[:, e, ni, 0:512],
                                                 start=first, stop=False)
                                nc.tensor.matmul(acc_b[:],
                                                 h_sb[:, e, ni, dsl],
                                                 wd_sb[:, e, ni, 512:1024],
                                                 start=first, stop=False)
                                first = False
                        nc.tensor.matmul(acc_a[:], hs_sb[:, dsl],
                                         wsd_sb[:, 0:512],
                                         start=False, stop=True)
                        nc.tensor.matmul(acc_b[:], hs_sb[:, dsl],
                                         wsd_sb[:, 512:1024],
                                         start=False, stop=True)
                        for nh, acc_ps in ((0, acc_a), (1, acc_b)):
                            hsl = slice(nh * 512, (nh + 1) * 512)
                            acc_sb = accs.tile([128, 512], BF, tag="accsb",
                                               name="acc_sb")
                            if nh == 0:
                                nc.scalar.copy(acc_sb[:], acc_ps[:])
                            else:
                                nc.vector.tensor_copy(acc_sb[:], acc_ps[:])
                            nc.sync.dma_start(out=acc_dram[chunk][lsl, hsl],
                                              in_=acc_sb[:])
                    nc.gpsimd.collective_compute(
                        "ReduceScatter", mybir.AluOpType.add, replica_groups=rg,
                        ins=[acc_dram[chunk].opt()], outs=[rs_out[chunk].opt()])

            # emission order = per-engine program order: the router vector
            # chain is sandwiched between expert blocks so its small
            # cross-engine chain never head-of-line-blocks DVE/ACT drains;
            # the router logits come after the first two gate/up blocks so
            # PE work starts on early-arriving bf16 weights while the fp32
            # X is still loading
            gu_block(None, 0)
            gu_block(0, 0)
            router_logits()
            router_transposes()
            router_chain()
            gu_block(1, 0)
            chain_tail()
            scale_block(0)
            down_block(0)
            gu_block(None, 1)
            gu_block(0, 1)
            gu_block(1, 1)
            scale_block(1)
            down_block(1)
            # final output: bf16 RS result -> SBUF -> fp32 cast -> out DRAM.
            # Emitted last so waiting on the RS results cannot block work.
            for chunk in range(NCHUNK):
                ob = accs.tile([shard, H], BF, tag="ob", name="ob")
                nc.sync.dma_start(out=ob[:], in_=rs_out[chunk])
                of = accs.tile([shard, H], F32, tag="of", name="of")
                nc.vector.tensor_copy(of[:], ob[:])
                nc.gpsimd.dma_start(
                    out=o[chunk * shard:(chunk + 1) * shard, :],
                    in_=of[:])

    nc.compile()
    return nc


_NC = None


def _get_nc():
    global _NC
    if _NC is None:
        _NC = build_nc()
    return _NC


def _make_in_maps(hidden_states, router_w, w_gate, w_up, w_down,
                  ws_gate_up, ws_down):
    xtf = np.ascontiguousarray(hidden_states.T.astype(np.float32))
    xtb = xtf.astype(BF16)
    maps = []
    for c in range(NCORES):
        own = [2 * c, 2 * c + 1]
        rest = [e for e in range(E) if e not in own]
        perm = own + rest
        rwt_t = router_w[perm].T.astype(np.float32)  # [H, E]
        rwt_c = np.ascontiguousarray(
            rwt_t.reshape(8, 128, E).transpose(1, 0, 2).reshape(128, 8 * E))
        gate_sl = ws_gate_up[:, c * ISH:(c + 1) * ISH]
        up_sl = ws_gate_up[:, E // 2 * ISH + c * ISH:E // 2 * ISH + (c + 1) * ISH]
        maps.append({
            "xtf": xtf,
            "xtb": xtb,
            "rwt": rwt_c,
            "wg": np.ascontiguousarray(w_gate[own]).astype(BF16),
            "wu": np.ascontiguousarray(w_up[own]).astype(BF16),
            "wd": np.ascontiguousarray(w_down[own]).astype(BF16),
            "wsgu": np.ascontiguousarray(
                np.concatenate([gate_sl, up_sl], axis=1)).astype(BF16),
            "wsd": np.ascontiguousarray(ws_down[c * ISH:(c + 1) * ISH]).astype(BF16),
        })
    return maps


def _assemble(results):
    shard = T // NCHUNK // NCORES
    out = np.empty((T, H), np.float32)
    for c in range(NCORES):
        oc = results[c]["o"]
        for h in range(NCHUNK):
            lo = h * (T // NCHUNK) + c * shard
            out[lo:lo + shard] = oc[h * shard:(h + 1) * shard]
    return out


def run(inputs, trace=False):
    """Run on hardware; returns (output, exec_time_ns or None)."""
    nc = _get_nc()
    maps = _make_in_maps(**inputs)
    res = run_bass_kernel_spmd(nc, maps, list(range(NCORES)), trace=trace)
    return _assemble(res.results), res.exec_time_ns


def kernel(**inputs):
    inputs = {k: np.asarray(v) for k, v in inputs.items()}
    out, _ = run(inputs, trace=False)
    return out



# revision 26
# speedup vs baseline: 1.0645x; 1.0645x over previous
"""BailingMoE block (router + 16 routed experts top-4 + shared SwiGLU MLP)
as a Trainium2 Bass/Tile kernel, expert-parallel over 8 NeuronCores.

Sharding:
  - Routed expert weight stacks [E,H,I] split along E: 2 experts per core
    (cast to bf16 on host; fp32 PSUM accumulation on device).
  - Shared-expert MLP tensor-parallel along the intermediate dim: 128 of
    1024 shared-intermediate channels per core.
  - Router replicated (fp32 - top-4 selection must match the reference);
    per-core the router weight columns are permuted so that each core's own
    2 experts land in columns 0/1 (softmax/top-k are permutation invariant).
  - Each core produces a full [T,H] partial (its experts + its shared
    slice); four bf16 ReduceScatters (256-token chunks, interleaved with
    the down-proj blocks so the collective overlaps compute) sum the
    partials on-device; each core casts its disjoint token slices back
    to fp32, and the host concatenates (pure unshard, no host math).
  - Router matmuls run as float32r (1 cycle/row on PE vs 4 for fp32);
    top-4 selection is bit-identical to fp32 for these logits.

Device dataflow (per core, all matmuls bf16 with fp32 accumulation):
  Xt = X^T staged [H,T];  G_t/U_t = Wg^T X^T per expert in [I,T] layout so
  both operands of every matmul are in their native layout (no transposes
  on the heavy path);  H = silu(G)*U*combine;  down-proj accumulates both
  experts + shared slice into one PSUM tile per (token-tile, out-half).
"""

import numpy as np
import ml_dtypes

import concourse.bass as bass
import concourse.mybir as mybir
import concourse.tile as tile
from concourse import bacc
from concourse.bass_utils import run_bass_kernel_spmd
from concourse.masks import make_identity

BF16 = ml_dtypes.bfloat16

NCORES = 8
T = 1024
H = 1024
I = 512  # routed expert intermediate
E = 16
TOP_K = 4
E_LOC = 2  # experts per core
ISH = 128  # shared-intermediate channels per core (1024 / 8)
KT = H // 128  # 8 contraction tiles over H
NI = I // 128  # 4 partition tiles over I
NTT = T // 128  # 8 token tiles
NCHUNK = 2  # ReduceScatter chunks

F32 = mybir.dt.float32
BF = mybir.dt.bfloat16


def build_nc():
    nc = bacc.Bacc("TRN2", target_bir_lowering=False, debug=False,
                   num_devices=NCORES)

    xtf = nc.dram_tensor("xtf", [H, T], mybir.dt.float32r, kind="ExternalInput")
    xtb = nc.dram_tensor("xtb", [H, T], BF, kind="ExternalInput")
    rwt = nc.dram_tensor("rwt", [128, KT * E], mybir.dt.float32r, kind="ExternalInput")
    wg = nc.dram_tensor("wg", [E_LOC, H, I], BF, kind="ExternalInput")
    wu = nc.dram_tensor("wu", [E_LOC, H, I], BF, kind="ExternalInput")
    wd = nc.dram_tensor("wd", [E_LOC, I, H], BF, kind="ExternalInput")
    wsgu = nc.dram_tensor("wsgu", [H, 2 * ISH], BF, kind="ExternalInput")
    wsd = nc.dram_tensor("wsd", [ISH, H], BF, kind="ExternalInput")
    o = nc.dram_tensor("o", [NCHUNK * (T // NCHUNK // NCORES), H], F32,
                       kind="ExternalOutput")

    rg = [list(range(NCORES))]

    with tile.TileContext(nc) as tc:
        with (
            tc.tile_pool(name="big", bufs=1) as big,
            tc.tile_pool(name="small", bufs=3) as small,
            tc.tile_pool(name="gs_pool", bufs=3) as gs_pool,
            tc.tile_pool(name="accs", bufs=3) as accs,
            tc.tile_pool(name="ps_small", bufs=1, space="PSUM") as ps_small,
            tc.tile_pool(name="ps_gu", bufs=2, space="PSUM") as ps_gu,
            tc.tile_pool(name="ps_acc", bufs=2, space="PSUM") as ps_acc,
            tc.tile_pool(name="dram", bufs=1, space="DRAM") as dram,
        ):
            # ---- staged inputs (everything fits in SBUF); DMAs chunked and
            # emitted in consumption order so compute starts early: the
            # shared-expert + expert-0 gate/up weights and X-bf16 come first
            # (first PE work), the fp32 X for the router after ----
            rwt_sb = big.tile([128, KT, E], mybir.dt.float32r)
            nc.sync.dma_start(out=rwt_sb, in_=rwt.rearrange("p (k e) -> p k e", e=E))
            wsgu_sb = big.tile([128, KT, 2 * ISH], BF)
            nc.sync.dma_start(out=wsgu_sb, in_=wsgu.rearrange("(k p) i -> p k i", p=128))
            xtb_r = xtb.rearrange("(k p) t -> p k t", p=128)
            xtb_sb = big.tile([128, KT, T], BF)
            for k in range(KT):
                nc.sync.dma_start(out=xtb_sb[:, k, :], in_=xtb_r[:, k, :])
            wg_sb = big.tile([128, E_LOC, KT, I], BF)
            wu_sb = big.tile([128, E_LOC, KT, I], BF)
            wg_r = wg.rearrange("e (k p) i -> p e k i", p=128)
            wu_r = wu.rearrange("e (k p) i -> p e k i", p=128)
            nc.sync.dma_start(out=wg_sb[:, 0], in_=wg_r[:, 0])
            nc.sync.dma_start(out=wu_sb[:, 0], in_=wu_r[:, 0])
            xtf_r = xtf.rearrange("(k p) t -> p k t", p=128)
            xtf_sb = big.tile([128, KT, T], mybir.dt.float32r)
            for k in range(KT):
                for hhalf in range(2):
                    hsl2 = slice(hhalf * 512, (hhalf + 1) * 512)
                    nc.sync.dma_start(out=xtf_sb[:, k, hsl2],
                                      in_=xtf_r[:, k, hsl2])
            nc.sync.dma_start(out=wg_sb[:, 1], in_=wg_r[:, 1])
            nc.sync.dma_start(out=wu_sb[:, 1], in_=wu_r[:, 1])
            wd_sb = big.tile([128, E_LOC, NI, H], BF)
            nc.sync.dma_start(out=wd_sb, in_=wd.rearrange("e (n p) h -> p e n h", p=128))
            wsd_sb = big.tile([128, H], BF)
            nc.sync.dma_start(out=wsd_sb, in_=wsd[:])

            identity = big.tile([128, 128], F32)
            make_identity(nc, identity)

            # one DRAM tensor per chunk: a shared tensor would put a false
            # WAR dependency between chunk k's RS read and chunk k+1's writes.
            # RS payload is bf16 (halves collective bytes); output cast back
            # to fp32 on-device after the RS.
            acc_dram = [dram.tile([T // NCHUNK, H], BF, name=f"acc_dram{i}")
                        for i in range(NCHUNK)]
            rs_out = dram.tile([NCHUNK, T // NCHUNK // NCORES, H], BF)
            c_scr = dram.tile([E_LOC, T], BF)


            # ---- router PE part: logits computed transposed ([E,T]:
            # 16 N=512 fp32 matmuls beat 64 N=16 ones), PE-transposed back
            # per token tile into [128, 8, 16] ----
            lgt_sb = small.tile([E, T], F32)

            def router_logits(th):
                tsl = slice(th * 512, (th + 1) * 512)
                lgt_ps = ps_small.tile([E, 512], F32, tag="sm", name="lgt_ps")
                for k in range(KT):
                    nc.tensor.matmul(lgt_ps[:], rwt_sb[:, k, :],
                                     xtf_sb[:, k, tsl],
                                     start=(k == 0), stop=(k == KT - 1))
                nc.scalar.copy(lgt_sb[:, tsl], lgt_ps[:])

            L = small.tile([128, NTT, E], F32)

            def router_transposes(th):
                for tt in range(th * 4, th * 4 + 4):
                    tr_ps = ps_acc.tile([128, E], F32, tag="acc", name="tr_ps")
                    nc.tensor.transpose(tr_ps[:],
                                        lgt_sb[:, tt * 128:(tt + 1) * 128],
                                        identity[0:E, 0:E])
                    nc.scalar.copy(L[:, tt, :], tr_ps[:])

            h_sb = big.tile([128, E_LOC, NI, T], BF)
            hs_sb = big.tile([128, T], BF)
            c_sb = big.tile([128, E_LOC, T], BF)
            shard = T // NCHUNK // NCORES  # 64

            def gu_block(e, th):
                """G/U projections + H = silu(G)*U for one expert/half
                (e is None -> shared-expert slice)."""
                tsl = slice(th * 512, (th + 1) * 512)
                for ni in range(NI if e is not None else 1):
                    g_ps = ps_gu.tile([128, 512], F32, tag="g", name="g_ps",
                                      bufs=2)
                    u_ps = ps_gu.tile([128, 512], F32, tag="u", name="u_ps")
                    if e is None:
                        wgl = wsgu_sb[:, :, 0:ISH]
                        wul = wsgu_sb[:, :, ISH:2 * ISH]
                        isl = slice(0, ISH)
                    else:
                        wgl = wg_sb[:, e]
                        wul = wu_sb[:, e]
                        isl = slice(ni * 128, (ni + 1) * 128)
                    for k in range(KT):
                        nc.tensor.matmul(g_ps[:], wgl[:, k, isl],
                                         xtb_sb[:, k, tsl],
                                         start=(k == 0), stop=(k == KT - 1))
                    for k in range(KT):
                        nc.tensor.matmul(u_ps[:], wul[:, k, isl],
                                         xtb_sb[:, k, tsl],
                                         start=(k == 0), stop=(k == KT - 1))
                    gs = gs_pool.tile([128, 512], BF, tag="gs", name="gs")
                    nc.scalar.activation(gs[:], g_ps[:],
                                         mybir.ActivationFunctionType.Silu)
                    dst = hs_sb[:, tsl] if e is None else h_sb[:, e, ni, tsl]
                    nc.vector.tensor_mul(dst, gs[:], u_ps[:])
                    if e is not None and th == 1:
                        # second half: the combine weights are long since
                        # ready - scale inline instead of a deferred pass
                        nc.vector.tensor_mul(dst, dst, c_sb[:, e, tsl])

            def router_chain(th):
                """Top-4 + normalized combine weights for one token half.
                Per-token-tile top-8 reductions (nc.vector.max) pipeline on
                DVE; splitting by half lets half-0's combine weights gate
                the first down-proj chunk ~a full logits+chain phase earlier."""
                t4 = slice(th * 4, th * 4 + 4)
                m = small.tile([128, 4, 8], F32, name=f"m{th}")
                msk = small.tile([128, 4, E], F32, name=f"msk{th}")
                for tt in range(th * 4, th * 4 + 4):
                    nc.vector.max(m[:, tt - th * 4, :], L[:, tt, :])
                nc.vector.tensor_tensor(
                    msk[:], L[:, t4, :],
                    m[:, :, 3:4].to_broadcast([128, 4, E]),
                    op=mybir.AluOpType.is_ge)
                nc.vector.tensor_tensor(
                    L[:, t4, :], L[:, t4, :],
                    m[:, :, 0:1].to_broadcast([128, 4, E]),
                    op=mybir.AluOpType.subtract)
                nc.scalar.activation(L[:, t4, :], L[:, t4, :],
                                     mybir.ActivationFunctionType.Exp)
                nc.vector.tensor_mul(L[:, t4, :], L[:, t4, :], msk[:])
                ssum = small.tile([128, 4, 1], F32, name=f"ssum{th}")
                nc.vector.reduce_sum(ssum[:, :, 0], L[:, t4, :],
                                     axis=mybir.AxisListType.X)
                rcp = small.tile([128, 4, 1], F32, name=f"rcp{th}")
                nc.vector.reciprocal(rcp[:, :, 0], ssum[:, :, 0])
                nc.vector.tensor_mul(L[:, t4, :], L[:, t4, :],
                                     rcp[:].to_broadcast([128, 4, E]))

            def chain_tail(th):
                # one PE transpose per half: [tok128, (tt e)] -> [(tt e), tok]
                t4 = slice(th * 4, th * 4 + 4)
                tsl = slice(th * 512, (th + 1) * 512)
                ct_ps = ps_small.tile([64, 128], F32, tag="ct", name=f"ct{th}")
                nc.tensor.transpose(
                    ct_ps[:], L[:, t4, :].rearrange("p t e -> p (t e)"),
                    identity[:])
                ct_sb = small.tile([64, 128], BF, name=f"ct_sb{th}")
                nc.scalar.copy(ct_sb[:], ct_ps[:])
                # per-expert combine rows broadcast to 128 partitions via a
                # DRAM round-trip (the DMA replicates the row)
                ct_v = ct_sb.rearrange("(t e) x -> t e x", e=E)
                for e in range(E_LOC):
                    nc.sync.dma_start(
                        out=c_scr[e, tsl].rearrange("(t x) -> t x", x=128),
                        in_=ct_v[:, e, :])
                    nc.sync.dma_start(
                        out=c_sb[:, e, tsl],
                        in_=c_scr[e:e + 1, tsl].to_broadcast([128, 512]))

            def scale_block(th):
                # deferred combine scale (first half only: its mul1s run
                # before the router chain finishes)
                if th == 1:
                    return
                tsl = slice(th * 512, (th + 1) * 512)
                for e in range(E_LOC):
                    for ni in range(NI):
                        nc.vector.tensor_mul(h_sb[:, e, ni, tsl],
                                             h_sb[:, e, ni, tsl],
                                             c_sb[:, e, tsl])

            def down_block(th):
                chunks_per_th = NCHUNK // 2
                tt_per_chunk = NTT // NCHUNK
                for cq in range(chunks_per_th):
                    chunk = th * chunks_per_th + cq
                    for ti in range(tt_per_chunk):
                        tt = chunk * tt_per_chunk + ti
                        dsl = slice(tt * 128, (tt + 1) * 128)
                        lsl = slice(ti * 128, (ti + 1) * 128)
                        # both output halves interleaved: each stationary
                        # H-tile load feeds two matmuls back-to-back
                        acc_a = ps_acc.tile([128, 512], F32, tag="acc",
                                            name="acc_a")
                        acc_b = ps_acc.tile([128, 512], F32, tag="acc",
                                            name="acc_b")
                        first = True
                        for e in range(E_LOC):
                            for ni in range(NI):
                                nc.tensor.matmul(acc_a[:],
                                                 h_sb[:, e, ni, dsl],
                                                 wd_sb[:, e, ni, 0:512],
                                                 start=first, stop=False)
                                nc.tensor.matmul(acc_b[:],
                                                 h_sb[:, e, ni, dsl],
                                                 wd_sb[:, e, ni, 512:1024],
                                                 start=first, stop=False)
                                first = False
                        nc.tensor.matmul(acc_a[:], hs_sb[:, dsl],
                                         wsd_sb[:, 0:512],
                                         start=False, stop=True)
                        nc.tensor.matmul(acc_b[:], hs_sb[:, dsl],
                                         wsd_sb[:, 512:1024],
                                         start=False, stop=True)
                        for nh, acc_ps in ((0, acc_a), (1, acc_b)):
                            hsl = slice(nh * 512, (nh + 1) * 512)
                            acc_sb = accs.tile([128, 512], BF, tag="accsb",
                                               name="acc_sb")
                            if nh == 0:
                                nc.scalar.copy(acc_sb[:], acc_ps[:])
                            else:
                                nc.vector.tensor_copy(acc_sb[:], acc_ps[:])
                            nc.sync.dma_start(out=acc_dram[chunk][lsl, hsl],
                                              in_=acc_sb[:])
                    nc.gpsimd.collective_compute(
                        "ReduceScatter", mybir.AluOpType.add, replica_groups=rg,
                        ins=[acc_dram[chunk].opt()], outs=[rs_out[chunk].opt()])

            # emission order = per-engine program order: the router vector
            # chain is sandwiched between expert blocks so its small
            # cross-engine chain never head-of-line-blocks DVE/ACT drains;
            # the router logits come after the first two gate/up blocks so
            # PE work starts on early-arriving bf16 weights while the fp32
            # X is still loading
            gu_block(None, 0)
            gu_block(0, 0)
            router_logits(0)
            router_transposes(0)
            router_logits(1)
            router_chain(0)
            router_transposes(1)
            gu_block(1, 0)
            chain_tail(0)
            router_chain(1)
            scale_block(0)
            down_block(0)
            chain_tail(1)
            gu_block(None, 1)
            gu_block(0, 1)
            gu_block(1, 1)
            scale_block(1)
            down_block(1)
            # final output: bf16 RS result -> SBUF -> fp32 cast -> out DRAM.
            # Emitted last so waiting on the RS results cannot block work.
            for chunk in range(NCHUNK):
                ob = accs.tile([shard, H], BF, tag="ob", name="ob")
                nc.sync.dma_start(out=ob[:], in_=rs_out[chunk])
                of = accs.tile([shard, H], F32, tag="of", name="of")
                nc.vector.tensor_copy(of[:], ob[:])
                nc.gpsimd.dma_start(
                    out=o[chunk * shard:(chunk + 1) * shard, :],
                    in_=of[:])

    nc.compile()
    return nc


_NC = None


def _get_nc():
    global _NC
    if _NC is None:
        _NC = build_nc()
    return _NC


def _make_in_maps(hidden_states, router_w, w_gate, w_up, w_down,
                  ws_gate_up, ws_down):
    xtf = np.ascontiguousarray(hidden_states.T.astype(np.float32))
    xtb = xtf.astype(BF16)
    maps = []
    for c in range(NCORES):
        own = [2 * c, 2 * c + 1]
        rest = [e for e in range(E) if e not in own]
        perm = own + rest
        rwt_t = router_w[perm].T.astype(np.float32)  # [H, E]
        rwt_c = np.ascontiguousarray(
            rwt_t.reshape(8, 128, E).transpose(1, 0, 2).reshape(128, 8 * E))
        gate_sl = ws_gate_up[:, c * ISH:(c + 1) * ISH]
        up_sl = ws_gate_up[:, E // 2 * ISH + c * ISH:E // 2 * ISH + (c + 1) * ISH]
        maps.append({
            "xtf": xtf,
            "xtb": xtb,
            "rwt": rwt_c,
            "wg": np.ascontiguousarray(w_gate[own]).astype(BF16),
            "wu": np.ascontiguousarray(w_up[own]).astype(BF16),
            "wd": np.ascontiguousarray(w_down[own]).astype(BF16),
            "wsgu": np.ascontiguousarray(
                np.concatenate([gate_sl, up_sl], axis=1)).astype(BF16),
            "wsd": np.ascontiguousarray(ws_down[c * ISH:(c + 1) * ISH]).astype(BF16),
        })
    return maps


def _assemble(results):
    shard = T // NCHUNK // NCORES
    out = np.empty((T, H), np.float32)
    for c in range(NCORES):
        oc = results[c]["o"]
        for h in range(NCHUNK):
            lo = h * (T // NCHUNK) + c * shard
            out[lo:lo + shard] = oc[h * shard:(h + 1) * shard]
    return out


def run(inputs, trace=False):
    """Run on hardware; returns (output, exec_time_ns or None)."""
    nc = _get_nc()
    maps = _make_in_maps(**inputs)
    res = run_bass_kernel_spmd(nc, maps, list(range(NCORES)), trace=trace)
    return _assemble(res.results), res.exec_time_ns


def kernel(**inputs):
    inputs = {k: np.asarray(v) for k, v in inputs.items()}
    out, _ = run(inputs, trace=False)
    return out

